# revision 11
# baseline (speedup 1.0000x reference)
"""Trainium2 Bass kernel for nn_DepthMemoryCache.

Reference computation (D=8, B=4, S=4096, C=1024, G=64):
    u     = einsum('bsc,gc->bsg', x[-1], W_u)
    keys  = einsum('dbc,gc->dbg', x.mean(2), W_u)
    gates = softmax(einsum('bsg,dbg->bsd', u, keys), axis=-1)
    out   = einsum('dbsc,bsd->bsc', x, gates)

Strategy: shard the sequence axis over 8 cores (core i gets
x[:, :, i*512:(i+1)*512, :]). Per core:

Phase A streams the 64MB shard once as 128 [128,1024] tiles through a
single SBUF ring. Each tile is cast to fp8-e4m3 (DVE/ACT alternating)
and column-summed by ONE DoubleRow matmul (K=256: the two c-halves ride
the two k-tiles, an indicator stationary routes half h of slab (d,b)
into psum row 2*(dB+b)+h), so the PE streams the whole shard at the
double-pumped fp8 rate. Depths 5,6,7 are additionally cast to resident
bf16 SBUF slabs. uT = W_u @ x7.T is computed on PE from the resident
bf16 x7 (transpose + matmul per c-chunk), interleaved across phase A.
The fixup (sums transpose -> keysT partials) runs in bf16, then an 8KB
AllReduce completes keys (a warm-up AllReduce at kernel start absorbs
the inter-core start skew; collective bounce DMAs ride GpSimd's queue).

Phase B shares the SAME tile ring: its per-block reads of the 5
non-resident depths queue directly behind phase A's reads in the DMA
rings, so the rings never drain across the collective. Per 128-row
block: one small logits matmul off resident uT, softmax via ACT exp
with accum_out + DVE reciprocal, then the depth-weighted sum as DVE
FMAs - resident depths first (bf16 sources), streamed depths (fp32) as
they land. y writes ride GpSimd (SWDGE).

HBM traffic per core: 64 (A) + 40 (B: 5/8 depths) + 8 (write) = 112MB.
fp8 only touches the gate path (means/logits ~2% -> gates ~2e-3);
resident bf16 touches 3/8 of the output sum (~2e-3); streamed depths
and the accumulator stay fp32.
"""
import sys

sys.path.insert(0, "/opt/trn_rl_repo")

from contextlib import ExitStack

import numpy as np
from concourse import bacc, bass, mybir, tile, masks
from concourse import bass_utils

F32 = mybir.dt.float32
BF16 = mybir.dt.bfloat16
F8 = mybir.dt.float8e4

D, B, S, C, G = 8, 4, 4096, 1024, 64
N_CORES = 8
P = 128                 # partition count / block rows
NKC = C // P            # 8 column chunks of 128
H = C // 2              # 512: c-half width (DoubleRow k-tile)
RESID = (7, 6, 5)       # depths resident in SBUF as bf16
POOL_BUFS = 14          # unified streaming ring


def build_body(tc, x, w, y, s_sh):
    """Emit the kernel IR. x:[D,B,s_sh,C], w:[G,C], y:[B,s_sh,C] dram APs."""
    nc = tc.nc
    nj = s_sh // P      # 128-row blocks per (d, b)
    mul, add = mybir.AluOpType.mult, mybir.AluOpType.add
    DB = D * B
    STREAMED = tuple(d for d in range(D) if d not in RESID)
    es = ExitStack()

    singles = es.enter_context(tc.tile_pool(name="singles", bufs=1))
    ident = singles.tile([P, P], F32)
    masks.make_identity(nc, ident[:])
    ident_bf = singles.tile([P, P], BF16)
    masks.make_identity(nc, ident_bf[:])
    # DoubleRow indicator: for slab r=(d*B+b), stationary ind_f8[:, r] is
    # [128, 2, 2*DB] with k-tile i routing c-half i into psum row 2r+i.
    ind_f8 = singles.tile([P, DB, 2, 2 * DB], F8)
    nc.vector.memset(ind_f8[:], 0.0)
    for r in range(DB):
        for i in range(2):
            nc.vector.memset(ind_f8[:, r, i, 2 * r + i:2 * r + i + 1], 1.0)
    xr = {}
    for d in RESID:
        xr[d] = singles.tile([P, B, nj, C], BF16, name=f"xr{d}")
    gates_sb = singles.tile([P, B, nj, D], F32)
    sums_sb = singles.tile([2 * DB, H], F32)        # row 2*(dB+b)+h
    sumk_sb = singles.tile([G, B * D], F32)
    meanT_bf = singles.tile([P, (NKC // 2) * 2 * DB], BF16)
    wT_bf = singles.tile([P, NKC, G], BF16)
    keysT_f32 = singles.tile([G, B * D], F32)
    keysT_bf = singles.tile([G, B, D], BF16)
    uT_bf = singles.tile([G, B, nj, P], BF16)

    # unified streaming ring: phase-A reads and phase-B reads share it, so
    # phase-B prefetch queues directly behind phase A in the DMA rings
    pool = es.enter_context(tc.tile_pool(name="pool", bufs=POOL_BUFS))
    f8p = es.enter_context(tc.tile_pool(name="f8p", bufs=3))
    accp = es.enter_context(tc.tile_pool(name="accp", bufs=2))
    small = es.enter_context(tc.tile_pool(name="small", bufs=4))

    dram = es.enter_context(tc.tile_pool(name="dram", bufs=1, space="DRAM"))
    # tiny warm-up AllReduce: absorbs collective-comm setup under phase A
    ccw_in = dram.tile([1, 16], F32)
    ccw_out = dram.tile([1, 16], F32)
    cc_in = dram.tile([G, B * D], F32)
    cc_out = dram.tile([G, B * D], F32)
    warm_sb = singles.tile([1, 16], F32)
    nc.vector.memset(warm_sb[:], 0.0)
    nc.gpsimd.dma_start(ccw_in[:], warm_sb[:])
    nc.gpsimd.collective_compute(
        "AllReduce", add, replica_groups=[list(range(N_CORES))],
        ins=[ccw_in.opt()], outs=[ccw_out.opt()],
    )

    # ---------------- Phase A: stream + fp8 DoubleRow sums ------------------
    with tc.tile_pool(name="psumS", bufs=1, space="PSUM") as psS, \
         tc.tile_pool(name="psumT", bufs=1, space="PSUM") as psT, \
         tc.tile_pool(name="psumXA", bufs=3, space="PSUM") as psXA, \
         tc.tile_pool(name="psumU", bufs=2, space="PSUM") as psU, \
         tc.tile_pool(name="xtA", bufs=3) as xtA, \
         tc.tile_pool(name="wpool", bufs=1) as wpool:
        sums_ps = psS.tile([2 * DB, H], F32)

        w_sb = wpool.tile([G, C], F32)
        nc.sync.dma_start(w_sb[:], w[:])
        # one-time W_u transpose: wT[c, g] chunks in bf16
        for k in range(NKC):
            tr = psT.tile([P, 2 * DB * (NKC // 2)], F32, tag="fix")
            nc.tensor.transpose(tr[:, :G], w_sb[:, k * P:(k + 1) * P],
                                ident[:G, :G])
            nc.scalar.copy(wT_bf[:, k, :], tr[:, :G])

        def u_block(b, j):
            # uT[g, s-block] = sum_k (wT_k).T @ x7T_k on PE (reads resident
            # bf16 x7, so this can run any time after the d=7 casts)
            u_ps = psU.tile([G, P], F32, tag="u")
            for k in range(NKC):
                xt_ps = psXA.tile([P, P], BF16, tag="xt_ps")
                nc.tensor.transpose(
                    xt_ps[:], xr[D - 1][:, b, j, k * P:(k + 1) * P],
                    ident_bf[:])
                xt_sb = xtA.tile([P, P], BF16, tag="xt_sb")
                if k % 2 == 0:
                    nc.scalar.copy(xt_sb[:], xt_ps[:])
                else:
                    nc.vector.tensor_copy(xt_sb[:], xt_ps[:])
                nc.tensor.matmul(
                    u_ps[:], wT_bf[:, k, :], xt_sb[:],
                    start=(k == 0), stop=(k == NKC - 1))
            nc.vector.tensor_copy(uT_bf[:, b, j, :], u_ps[:])

        ublocks = [(b, j) for b in range(B) for j in range(nj)]
        ub_i = 0
        ti = 0
        NT = DB * nj
        for dd in range(D):
            d = (dd + D - 1) % D        # d = 7 first (fills resident x7)
            for b in range(B):
                r = d * B + b
                for j in range(nj):
                    t = pool.tile([P, C], F32, tag="t")
                    nc.sync.dma_start(t[:], x[d, b, j * P:(j + 1) * P, :])
                    q = f8p.tile([P, C], F8, tag="q")
                    if ti % 2 == 0:
                        nc.vector.tensor_copy(q[:], t[:])
                        if d in RESID:
                            nc.scalar.copy(xr[d][:, b, j, :], t[:])
                    else:
                        nc.scalar.copy(q[:], t[:])
                        if d in RESID:
                            nc.vector.tensor_copy(xr[d][:, b, j, :], t[:])
                    nc.tensor.matmul(
                        sums_ps[:], ind_f8[:, r],
                        q[:].rearrange("p (i h) -> p i h", i=2),
                        start=(ti == 0), stop=(ti == NT - 1),
                        perf_mode=mybir.MatmulPerfMode.DoubleRow)
                    if dd >= 1 and ti % 4 == 0 and ub_i < len(ublocks):
                        ub, uj = ublocks[ub_i]
                        ub_i += 1
                        u_block(ub, uj)
                    ti += 1
        while ub_i < len(ublocks):
            ub, uj = ublocks[ub_i]
            ub_i += 1
            u_block(ub, uj)

        # raw sums -> sbuf with the 1/S mean scale folded in (ACT)
        nc.scalar.mul(sums_sb[:], sums_ps[:], 1.0 / S)

        # ---- local partial keysT in bf16 (keys are linear in the means, ----
        # ---- so the AllReduce runs in the tiny keys space: 8KB)         ----
        # meanT chunks: 4 fp32 transposes [64,128] -> [128,64] into one psum
        mt_ps = psT.tile([P, 2 * DB * (NKC // 2)], F32, tag="fix")
        for kp in range(NKC // 2):
            nc.tensor.matmul(
                mt_ps[:, kp * 2 * DB:(kp + 1) * 2 * DB],
                sums_sb[:, kp * P:(kp + 1) * P], ident[:2 * DB, :2 * DB],
                is_transpose=True,
                start=(kp == 0), stop=(kp == NKC // 2 - 1))
        nc.vector.tensor_copy(meanT_bf[:], mt_ps[:])
        # partial keysT[g, d] per b = sum_(h,k') wT_(h*4+k').T @ meanT cols
        keys_ps = psT.tile([P, 2 * DB * (NKC // 2)], F32, tag="fix")
        for bb in range(B):
            for kk in range(NKC):
                h, kp = divmod(kk, NKC // 2)
                mcols = meanT_bf[:, kp * 2 * DB:(kp + 1) * 2 * DB].rearrange(
                    "p (dd m) -> p dd m", m=2 * B)[:, :, 2 * bb + h]
                nc.tensor.matmul(
                    keys_ps[:G, bb * D:(bb + 1) * D],
                    wT_bf[:, kk, :], mcols,
                    start=(kk == 0), stop=(kk == NKC - 1))
        nc.vector.tensor_copy(sumk_sb[:], keys_ps[:G, :B * D])

    # ---------------- AllReduce the [G, B*D] partial keys -------------------
    # bounce DMAs go through GpSimd's queue so the Sync engine never blocks
    # on the collective and keeps issuing phase-B prefetch reads.
    nc.gpsimd.dma_start(cc_in[:], sumk_sb[:])
    nc.gpsimd.collective_compute(
        "AllReduce", add,
        replica_groups=[list(range(N_CORES))],
        ins=[cc_in.opt()], outs=[cc_out.opt()],
    )
    nc.gpsimd.dma_start(keysT_f32[:], cc_out[:])
    nc.scalar.copy(keysT_bf[:].rearrange("g b d -> g (b d)"), keysT_f32[:])

    # ---------------- Phase B: gates + depth-weighted sum -------------------
    # Resident depths ride the PE: ACT builds diag(g_d) stationaries, PE
    # accumulates g_d * x_d for d in RESID into PSUM. DVE chains the
    # streamed depths (freeing pool slots ASAP post-collective) and merges
    # the PSUM partial at the end.
    with tc.tile_pool(name="psumL", bufs=2, space="PSUM") as psL, \
         tc.tile_pool(name="psumAcc", bufs=2, space="PSUM") as psA, \
         tc.tile_pool(name="diagp", bufs=3) as diagp:
        for b in range(B):
            for j in range(nj):
                ts = {}
                for sd in STREAMED:
                    tt = pool.tile([P, C], F32, tag="t")
                    nc.sync.dma_start(tt[:], x[sd, b, j * P:(j + 1) * P, :])
                    ts[sd] = tt
                # logits for this block: one small matmul off resident uT
                lg_ps = psL.tile([P, D], F32, tag="lg")
                nc.tensor.matmul(lg_ps[:], uT_bf[:, b, j, :],
                                 keysT_bf[:, b, :])
                e_sb = small.tile([P, D], F32, tag="e")
                z_sb = small.tile([P, 1], F32, tag="z")
                rz_sb = small.tile([P, 1], F32, tag="rz")
                nc.scalar.activation(
                    e_sb[:], lg_ps[:], mybir.ActivationFunctionType.Exp,
                    accum_out=z_sb[:])
                nc.vector.reciprocal(rz_sb[:], z_sb[:])
                nc.scalar.mul(gates_sb[:, b, j, :], e_sb[:], rz_sb[:])

                # resident partial on PE
                r_ps = psA.tile([P, C], F32, tag="racc")
                for di, d in enumerate(RESID):
                    dg = diagp.tile([P, P], BF16, tag="dg")
                    nc.scalar.mul(dg[:], ident_bf[:],
                                  gates_sb[:, b, j, d:d + 1])
                    for h in range(2):
                        nc.tensor.matmul(
                            r_ps[:, h * H:(h + 1) * H], dg[:],
                            xr[d][:, b, j, h * H:(h + 1) * H],
                            start=(di == 0), stop=(di == len(RESID) - 1))
                # streamed chain on DVE, psum partial merged at the end
                acc = accp.tile([P, C], F32, tag="acc")
                for si, d in enumerate(STREAMED):
                    g = gates_sb[:, b, j, d:d + 1]
                    if si == 0:
                        nc.vector.tensor_scalar_mul(acc[:], ts[d][:], g)
                    else:
                        nc.vector.scalar_tensor_tensor(
                            out=acc[:], in0=ts[d][:], scalar=g,
                            in1=acc[:], op0=mul, op1=add)
                nc.vector.scalar_tensor_tensor(
                    out=acc[:], in0=r_ps[:], scalar=1.0,
                    in1=acc[:], op0=mul, op1=add)
                # y writes via GpSimd (SWDGE): keeps Sync's in-order queue
                # free for prefetch reads
                nc.gpsimd.dma_start(y[b, j * P:(j + 1) * P, :], acc[:])

    es.close()


def build_nc(s_sh):
    nc = bacc.Bacc("TRN2", target_bir_lowering=False, debug=False,
                   num_devices=N_CORES)
    x_ap = nc.dram_tensor("x", [D, B, s_sh, C], F32, kind="ExternalInput").ap()
    w_ap = nc.dram_tensor("w", [G, C], F32, kind="ExternalInput").ap()
    y_ap = nc.dram_tensor("y", [B, s_sh, C], F32, kind="ExternalOutput").ap()
    with tile.TileContext(nc) as tc:
        build_body(tc, x_ap, w_ap, y_ap, s_sh)
    nc.compile()
    return nc


_NC_CACHE = {}


def _get_nc(s_sh):
    if s_sh not in _NC_CACHE:
        _NC_CACHE[s_sh] = build_nc(s_sh)
    return _NC_CACHE[s_sh]


def run(cached_states, W_u, trace=False, trace_cores=None):
    s_sh = S // N_CORES
    nc = _get_nc(s_sh)
    xs = np.asarray(cached_states, dtype=np.float32)
    ws = np.ascontiguousarray(np.asarray(W_u, dtype=np.float32))
    in_maps = []
    for i in range(N_CORES):
        sh = np.ascontiguousarray(xs[:, :, i * s_sh:(i + 1) * s_sh, :])
        in_maps.append({"x": sh, "w": ws})
    res = bass_utils.run_bass_kernel_spmd(
        nc, in_maps, core_ids=list(range(N_CORES)), trace=trace,
        trace_cores=trace_cores)
    out = np.empty((B, S, C), np.float32)
    for i in range(N_CORES):
        out[:, i * s_sh:(i + 1) * s_sh, :] = res.results[i]["y"]
    return out, res


def kernel(cached_states, W_u):
    out, _ = run(cached_states, W_u)
    return out


# revision 16
# speedup vs baseline: 1.0345x; 1.0345x over previous
"""Trainium2 Bass kernel for nn_DepthMemoryCache.

Reference computation (D=8, B=4, S=4096, C=1024, G=64):
    u     = einsum('bsc,gc->bsg', x[-1], W_u)
    keys  = einsum('dbc,gc->dbg', x.mean(2), W_u)
    gates = softmax(einsum('bsg,dbg->bsd', u, keys), axis=-1)
    out   = einsum('dbsc,bsd->bsc', x, gates)

Strategy: shard the sequence axis over 8 cores (core i gets
x[:, :, i*512:(i+1)*512, :]). Per core:

Phase A streams the 64MB shard once as 128 [128,1024] tiles through a
single SBUF ring. Each tile is cast to fp8-e4m3 (DVE/ACT alternating)
and column-summed by ONE DoubleRow matmul (K=256: the two c-halves ride
the two k-tiles, an indicator stationary routes half h of slab (d,b)
into psum row 2*(dB+b)+h), so the PE streams the whole shard at the
double-pumped fp8 rate. Depths 5,6,7 are additionally cast to resident
bf16 SBUF slabs. uT = W_u @ x7.T is computed on PE from the resident
bf16 x7 (transpose + matmul per c-chunk), interleaved across phase A.
The fixup (sums transpose -> keysT partials) runs in bf16, then an 8KB
AllReduce completes keys (a warm-up AllReduce at kernel start absorbs
the inter-core start skew; collective bounce DMAs ride GpSimd's queue).

Phase B shares the SAME tile ring: its per-block reads of the 5
non-resident depths queue directly behind phase A's reads in the DMA
rings, so the rings never drain across the collective. Per 128-row
block: one small logits matmul off resident uT, softmax via ACT exp
with accum_out + DVE reciprocal, then the depth-weighted sum as DVE
FMAs - resident depths first (bf16 sources), streamed depths (fp32) as
they land. y writes ride GpSimd (SWDGE).

HBM traffic per core: 64 (A) + 40 (B: 5/8 depths) + 8 (write) = 112MB.
fp8 only touches the gate path (means/logits ~2% -> gates ~2e-3);
resident bf16 touches 3/8 of the output sum (~2e-3); streamed depths
and the accumulator stay fp32.
"""
import sys

sys.path.insert(0, "/opt/trn_rl_repo")

from contextlib import ExitStack

import numpy as np
from concourse import bacc, bass, mybir, tile, masks
from concourse import bass_utils

F32 = mybir.dt.float32
BF16 = mybir.dt.bfloat16
F8 = mybir.dt.float8e4

D, B, S, C, G = 8, 4, 4096, 1024, 64
N_CORES = 8
P = 128                 # partition count / block rows
NKC = C // P            # 8 column chunks of 128
H = C // 2              # 512: c-half width (DoubleRow k-tile)
RESID = (7, 6, 5)       # depths resident in SBUF as bf16
POOL_BUFS = 14          # unified streaming ring


def build_body(tc, x, w, y, s_sh):
    """Emit the kernel IR. x:[D,B,s_sh,C], w:[G,C], y:[B,s_sh,C] dram APs."""
    nc = tc.nc
    nj = s_sh // P      # 128-row blocks per (d, b)
    mul, add = mybir.AluOpType.mult, mybir.AluOpType.add
    DB = D * B
    STREAMED = tuple(d for d in range(D) if d not in RESID)
    es = ExitStack()

    singles = es.enter_context(tc.tile_pool(name="singles", bufs=1))
    ident = singles.tile([P, P], F32)
    masks.make_identity(nc, ident[:])
    ident_bf = singles.tile([P, P], BF16)
    masks.make_identity(nc, ident_bf[:])
    # DoubleRow indicator: for slab r=(d*B+b), stationary ind_f8[:, r] is
    # [128, 2, 2*DB] with k-tile i routing c-half i into psum row 2r+i.
    ind_f8 = singles.tile([P, DB, 2, 2 * DB], F8)
    nc.vector.memset(ind_f8[:], 0.0)
    for r in range(DB):
        for i in range(2):
            nc.vector.memset(ind_f8[:, r, i, 2 * r + i:2 * r + i + 1], 1.0)
    ident_f8 = singles.tile([P, P], F8)
    masks.make_identity(nc, ident_f8[:])
    xr = {}
    for d in RESID:
        xr[d] = singles.tile([P, B, nj, C], BF16, name=f"xr{d}")
    gates_sb = singles.tile([P, B, nj, D], F32)
    sums_sb = singles.tile([2 * DB, H], F32)        # row 2*(dB+b)+h
    sumk_sb = singles.tile([G, B * D], F32)
    meanT_bf = singles.tile([P, (NKC // 2) * 2 * DB], BF16)
    wT_bf = singles.tile([P, NKC, G], BF16)
    wT_f8 = singles.tile([P, NKC // 2, 2, G], F8)
    keysT_f32 = singles.tile([G, B * D], F32)
    keysT_bf = singles.tile([G, B, D], BF16)
    uT_bf = singles.tile([G, B, nj, P], BF16)

    # unified streaming ring: phase-A reads and phase-B reads share it, so
    # phase-B prefetch queues directly behind phase A in the DMA rings
    pool = es.enter_context(tc.tile_pool(name="pool", bufs=POOL_BUFS))
    f8p = es.enter_context(tc.tile_pool(name="f8p", bufs=3))
    accp = es.enter_context(tc.tile_pool(name="accp", bufs=2))
    small = es.enter_context(tc.tile_pool(name="small", bufs=4))

    dram = es.enter_context(tc.tile_pool(name="dram", bufs=1, space="DRAM"))
    # tiny warm-up AllReduce: absorbs collective-comm setup under phase A
    ccw_in = dram.tile([1, 16], F32)
    ccw_out = dram.tile([1, 16], F32)
    cc_in = dram.tile([G, B * D], F32)
    cc_out = dram.tile([G, B * D], F32)
    warm_sb = singles.tile([1, 16], F32)
    nc.vector.memset(warm_sb[:], 0.0)
    nc.gpsimd.dma_start(ccw_in[:], warm_sb[:])
    nc.gpsimd.collective_compute(
        "AllReduce", add, replica_groups=[list(range(N_CORES))],
        ins=[ccw_in.opt()], outs=[ccw_out.opt()],
    )

    # ---------------- Phase A: stream + fp8 DoubleRow sums ------------------
    with tc.tile_pool(name="psumS", bufs=1, space="PSUM") as psS, \
         tc.tile_pool(name="psumT", bufs=1, space="PSUM") as psT, \
         tc.tile_pool(name="psumXA", bufs=3, space="PSUM") as psXA, \
         tc.tile_pool(name="psumU", bufs=2, space="PSUM") as psU, \
         tc.tile_pool(name="xtA", bufs=3) as xtA, \
         tc.tile_pool(name="wpool", bufs=1) as wpool:
        sums_ps = psS.tile([2 * DB, H], F32)

        w_sb = wpool.tile([G, C], F32)
        nc.sync.dma_start(w_sb[:], w[:])
        # one-time W_u transpose: wT[c, g] chunks in bf16 (fixup) + fp8 (u)
        for k in range(NKC):
            tr = psT.tile([P, 2 * DB * (NKC // 2)], F32, tag="fix")
            nc.tensor.transpose(tr[:, :G], w_sb[:, k * P:(k + 1) * P],
                                ident[:G, :G])
            nc.scalar.copy(wT_bf[:, k, :], tr[:, :G])
            nc.vector.tensor_copy(wT_f8[:, k // 2, k % 2, :], tr[:, :G])

        # phase-A tile order: d7 tiles (which carry the u-path PE work)
        # spread 1-in-7 among the d0..d6 stream so PE load stays level
        others = [(d_, b_, j_) for d_ in range(D - 1)
                  for b_ in range(B) for j_ in range(nj)]
        d7s = [(D - 1, b_, j_) for b_ in range(B) for j_ in range(nj)]
        NT = DB * nj
        seq = []
        oi = si = 0
        for i in range(NT):
            if i % 7 == 3 and si < len(d7s):
                seq.append(d7s[si])
                si += 1
            else:
                seq.append(others[oi])
                oi += 1

        for ti, (d, b, j) in enumerate(seq):
            r = d * B + b
            t = pool.tile([P, C], F32, tag="t")
            nc.sync.dma_start(t[:], x[d, b, j * P:(j + 1) * P, :])
            q = f8p.tile([P, C], F8, tag="q")
            # casts split ~4:3 DVE:ACT (ACT's converter is slower)
            on_dve = ti % 7 in (0, 2, 4, 6)
            if on_dve:
                nc.vector.tensor_copy(q[:], t[:])
                if d in RESID:
                    nc.scalar.copy(xr[d][:, b, j, :], t[:])
            else:
                nc.scalar.copy(q[:], t[:])
                if d in RESID:
                    nc.vector.tensor_copy(xr[d][:, b, j, :], t[:])
            nc.tensor.matmul(
                sums_ps[:], ind_f8[:, r],
                q[:].rearrange("p (i h) -> p i h", i=2),
                start=(ti == 0), stop=(ti == NT - 1),
                perf_mode=mybir.MatmulPerfMode.DoubleRow)
            if d == D - 1:
                # u-path for block (b, j), all fp8: transpose the fp8 tile's
                # c-chunks on PE, DoubleRow uT += wT_f8.T @ x7T pairs
                u_ps = psU.tile([G, P], F32, tag="u")
                for kp in range(NKC // 2):
                    xt2 = xtA.tile([P, 2, P], F8, tag="xt2")
                    for i2 in range(2):
                        k = 2 * kp + i2
                        # fp8 transpose writes with element step 2
                        xt_ps = psXA.tile([P, P, 2], F8, tag="xt_ps")
                        nc.tensor.matmul(
                            xt_ps[:, :, 0], q[:, k * P:(k + 1) * P],
                            ident_f8[:], is_transpose=True)
                        if (kp + i2) % 2 == 0:
                            nc.scalar.copy(xt2[:, i2, :], xt_ps[:, :, 0])
                        else:
                            nc.vector.tensor_copy(xt2[:, i2, :],
                                                  xt_ps[:, :, 0])
                    nc.tensor.matmul(
                        u_ps[:], wT_f8[:, kp], xt2[:],
                        start=(kp == 0), stop=(kp == NKC // 2 - 1),
                        perf_mode=mybir.MatmulPerfMode.DoubleRow)
                nc.vector.tensor_copy(uT_bf[:, b, j, :], u_ps[:])

        # raw sums -> sbuf with the 1/S mean scale folded in (ACT)
        nc.scalar.mul(sums_sb[:], sums_ps[:], 1.0 / S)

        # ---- local partial keysT in bf16 (keys are linear in the means, ----
        # ---- so the AllReduce runs in the tiny keys space: 8KB)         ----
        # meanT chunks: 4 fp32 transposes [64,128] -> [128,64] into one psum
        mt_ps = psT.tile([P, 2 * DB * (NKC // 2)], F32, tag="fix")
        for kp in range(NKC // 2):
            nc.tensor.matmul(
                mt_ps[:, kp * 2 * DB:(kp + 1) * 2 * DB],
                sums_sb[:, kp * P:(kp + 1) * P], ident[:2 * DB, :2 * DB],
                is_transpose=True,
                start=(kp == 0), stop=(kp == NKC // 2 - 1))
        nc.vector.tensor_copy(meanT_bf[:], mt_ps[:])
        # partial keysT[g, d] per b = sum_(h,k') wT_(h*4+k').T @ meanT cols
        keys_ps = psT.tile([P, 2 * DB * (NKC // 2)], F32, tag="fix")
        for bb in range(B):
            for kk in range(NKC):
                h, kp = divmod(kk, NKC // 2)
                mcols = meanT_bf[:, kp * 2 * DB:(kp + 1) * 2 * DB].rearrange(
                    "p (dd m) -> p dd m", m=2 * B)[:, :, 2 * bb + h]
                nc.tensor.matmul(
                    keys_ps[:G, bb * D:(bb + 1) * D],
                    wT_bf[:, kk, :], mcols,
                    start=(kk == 0), stop=(kk == NKC - 1))
        nc.vector.tensor_copy(sumk_sb[:], keys_ps[:G, :B * D])

    # ---------------- AllReduce the [G, B*D] partial keys -------------------
    # bounce DMAs go through GpSimd's queue so the Sync engine never blocks
    # on the collective and keeps issuing phase-B prefetch reads.
    nc.gpsimd.dma_start(cc_in[:], sumk_sb[:])
    nc.gpsimd.collective_compute(
        "AllReduce", add,
        replica_groups=[list(range(N_CORES))],
        ins=[cc_in.opt()], outs=[cc_out.opt()],
    )
    nc.gpsimd.dma_start(keysT_f32[:], cc_out[:])
    nc.vector.tensor_copy(keysT_bf[:].rearrange("g b d -> g (b d)"),
                          keysT_f32[:])

    # ---------------- Phase B: gates + depth-weighted sum -------------------
    # Resident depths ride the PE: ACT builds diag(g_d) stationaries, PE
    # accumulates g_d * x_d for d in RESID into PSUM. DVE chains the
    # streamed depths (freeing pool slots ASAP post-collective) and merges
    # the PSUM partial at the end.
    with tc.tile_pool(name="psumL", bufs=2, space="PSUM") as psL, \
         tc.tile_pool(name="psumAcc", bufs=2, space="PSUM") as psA, \
         tc.tile_pool(name="diagp", bufs=3) as diagp:
        for b in range(B):
            for j in range(nj):
                ts = {}
                for sd in STREAMED:
                    tt = pool.tile([P, C], F32, tag="t")
                    nc.sync.dma_start(tt[:], x[sd, b, j * P:(j + 1) * P, :])
                    ts[sd] = tt
                # logits for this block: one small matmul off resident uT
                lg_ps = psL.tile([P, D], F32, tag="lg")
                nc.tensor.matmul(lg_ps[:], uT_bf[:, b, j, :],
                                 keysT_bf[:, b, :])
                e_sb = small.tile([P, D], F32, tag="e")
                z_sb = small.tile([P, 1], F32, tag="z")
                rz_sb = small.tile([P, 1], F32, tag="rz")
                nc.scalar.activation(
                    e_sb[:], lg_ps[:], mybir.ActivationFunctionType.Exp,
                    accum_out=z_sb[:])
                nc.vector.reciprocal(rz_sb[:], z_sb[:])
                nc.scalar.mul(gates_sb[:, b, j, :], e_sb[:], rz_sb[:])

                # resident partial on PE
                r_ps = psA.tile([P, C], F32, tag="racc")
                for di, d in enumerate(RESID):
                    dg = diagp.tile([P, P], BF16, tag="dg")
                    nc.scalar.mul(dg[:], ident_bf[:],
                                  gates_sb[:, b, j, d:d + 1])
                    for h in range(2):
                        nc.tensor.matmul(
                            r_ps[:, h * H:(h + 1) * H], dg[:],
                            xr[d][:, b, j, h * H:(h + 1) * H],
                            start=(di == 0), stop=(di == len(RESID) - 1))
                # streamed chain on DVE, psum partial merged at the end
                acc = accp.tile([P, C], F32, tag="acc")
                for si, d in enumerate(STREAMED):
                    g = gates_sb[:, b, j, d:d + 1]
                    if si == 0:
                        nc.vector.tensor_scalar_mul(acc[:], ts[d][:], g)
                    else:
                        nc.vector.scalar_tensor_tensor(
                            out=acc[:], in0=ts[d][:], scalar=g,
                            in1=acc[:], op0=mul, op1=add)
                nc.vector.scalar_tensor_tensor(
                    out=acc[:], in0=r_ps[:], scalar=1.0,
                    in1=acc[:], op0=mul, op1=add)
                # y writes via GpSimd (SWDGE): keeps Sync's in-order queue
                # free for prefetch reads
                nc.gpsimd.dma_start(y[b, j * P:(j + 1) * P, :], acc[:])

    es.close()


def build_nc(s_sh):
    nc = bacc.Bacc("TRN2", target_bir_lowering=False, debug=False,
                   num_devices=N_CORES)
    x_ap = nc.dram_tensor("x", [D, B, s_sh, C], F32, kind="ExternalInput").ap()
    w_ap = nc.dram_tensor("w", [G, C], F32, kind="ExternalInput").ap()
    y_ap = nc.dram_tensor("y", [B, s_sh, C], F32, kind="ExternalOutput").ap()
    with tile.TileContext(nc) as tc:
        build_body(tc, x_ap, w_ap, y_ap, s_sh)
    nc.compile()
    return nc


_NC_CACHE = {}


def _get_nc(s_sh):
    if s_sh not in _NC_CACHE:
        _NC_CACHE[s_sh] = build_nc(s_sh)
    return _NC_CACHE[s_sh]


def run(cached_states, W_u, trace=False, trace_cores=None):
    s_sh = S // N_CORES
    nc = _get_nc(s_sh)
    xs = np.asarray(cached_states, dtype=np.float32)
    ws = np.ascontiguousarray(np.asarray(W_u, dtype=np.float32))
    in_maps = []
    for i in range(N_CORES):
        sh = np.ascontiguousarray(xs[:, :, i * s_sh:(i + 1) * s_sh, :])
        in_maps.append({"x": sh, "w": ws})
    res = bass_utils.run_bass_kernel_spmd(
        nc, in_maps, core_ids=list(range(N_CORES)), trace=trace,
        trace_cores=trace_cores)
    out = np.empty((B, S, C), np.float32)
    for i in range(N_CORES):
        out[:, i * s_sh:(i + 1) * s_sh, :] = res.results[i]["y"]
    return out, res


def kernel(cached_states, W_u):
    out, _ = run(cached_states, W_u)
    return out


# revision 19
# speedup vs baseline: 1.0788x; 1.0428x over previous
"""Trainium2 Bass kernel for nn_DepthMemoryCache.

Reference computation (D=8, B=4, S=4096, C=1024, G=64):
    u     = einsum('bsc,gc->bsg', x[-1], W_u)
    keys  = einsum('dbc,gc->dbg', x.mean(2), W_u)
    gates = softmax(einsum('bsg,dbg->bsd', u, keys), axis=-1)
    out   = einsum('dbsc,bsd->bsc', x, gates)

Strategy: shard the sequence axis over 8 cores (core i gets
x[:, :, i*512:(i+1)*512, :]). Per core:

Phase A streams the 64MB shard once as 128 [128,1024] tiles through a
single SBUF ring. Each tile is cast to fp8-e4m3 (DVE/ACT alternating)
and column-summed by ONE DoubleRow matmul (K=256: the two c-halves ride
the two k-tiles, an indicator stationary routes half h of slab (d,b)
into psum row 2*(dB+b)+h), so the PE streams the whole shard at the
double-pumped fp8 rate. Depths 5,6,7 are additionally cast to resident
bf16 SBUF slabs. uT = W_u @ x7.T is computed on PE from the resident
bf16 x7 (transpose + matmul per c-chunk), interleaved across phase A.
The fixup (sums transpose -> keysT partials) runs in bf16, then an 8KB
AllReduce completes keys (a warm-up AllReduce at kernel start absorbs
the inter-core start skew; collective bounce DMAs ride GpSimd's queue).

Phase B shares the SAME tile ring: its per-block reads of the 5
non-resident depths queue directly behind phase A's reads in the DMA
rings, so the rings never drain across the collective. Per 128-row
block: one small logits matmul off resident uT, softmax via ACT exp
with accum_out + DVE reciprocal, then the depth-weighted sum as DVE
FMAs - resident depths first (bf16 sources), streamed depths (fp32) as
they land. y writes ride GpSimd (SWDGE).

HBM traffic per core: 64 (A) + 40 (B: 5/8 depths) + 8 (write) = 112MB.
fp8 only touches the gate path (means/logits ~2% -> gates ~2e-3);
resident bf16 touches 3/8 of the output sum (~2e-3); streamed depths
and the accumulator stay fp32.
"""
import sys

sys.path.insert(0, "/opt/trn_rl_repo")

from contextlib import ExitStack

import numpy as np
from concourse import bacc, bass, mybir, tile, masks
from concourse import bass_utils

F32 = mybir.dt.float32
BF16 = mybir.dt.bfloat16
F8 = mybir.dt.float8e4

D, B, S, C, G = 8, 4, 4096, 1024, 64
N_CORES = 8
P = 128                 # partition count / block rows
NKC = C // P            # 8 column chunks of 128
H = C // 2              # 512: c-half width (DoubleRow k-tile)
RESID = (7, 6, 5, 4)    # depths resident in SBUF as bf16
POOL_BUFS = 10          # unified streaming ring


def build_body(tc, x, w, y, s_sh):
    """Emit the kernel IR. x:[D,B,s_sh,C], w:[G,C], y:[B,s_sh,C] dram APs."""
    nc = tc.nc
    nj = s_sh // P      # 128-row blocks per (d, b)
    mul, add = mybir.AluOpType.mult, mybir.AluOpType.add
    DB = D * B
    STREAMED = tuple(d for d in range(D) if d not in RESID)
    es = ExitStack()

    singles = es.enter_context(tc.tile_pool(name="singles", bufs=1))
    ident = singles.tile([P, P], F32)
    masks.make_identity(nc, ident[:])
    ident_bf = singles.tile([P, P], BF16)
    masks.make_identity(nc, ident_bf[:])
    # DoubleRow indicator: for slab r=(d*B+b), stationary ind_f8[:, r] is
    # [128, 2, 2*DB] with k-tile i routing c-half i into psum row 2r+i.
    ind_f8 = singles.tile([P, DB, 2, 2 * DB], F8)
    nc.vector.memset(ind_f8[:], 0.0)
    for r in range(DB):
        for i in range(2):
            nc.vector.memset(ind_f8[:, r, i, 2 * r + i:2 * r + i + 1], 1.0)
    ident_f8 = singles.tile([P, P], F8)
    masks.make_identity(nc, ident_f8[:])
    xr = {}
    for d in RESID:
        xr[d] = singles.tile([P, B, nj, C], BF16, name=f"xr{d}")
    gates_sb = singles.tile([P, B, nj, D], F32)
    sums_sb = singles.tile([2 * DB, H], F32)        # row 2*(dB+b)+h
    sumk_sb = singles.tile([G, B * D], F32)
    meanT_bf = singles.tile([P, (NKC // 2) * 2 * DB], BF16)
    wT_bf = singles.tile([P, NKC, G], BF16)
    wT_f8 = singles.tile([P, NKC // 2, 2, G], F8)
    keysT_f32 = singles.tile([G, B * D], F32)
    keysT_bf = singles.tile([G, B, D], BF16)
    uT_bf = singles.tile([G, B, nj, P], BF16)

    # unified streaming ring: phase-A reads and phase-B reads share it, so
    # phase-B prefetch queues directly behind phase A in the DMA rings
    pool = es.enter_context(tc.tile_pool(name="pool", bufs=POOL_BUFS))
    f8p = es.enter_context(tc.tile_pool(name="f8p", bufs=2))
    accp = es.enter_context(tc.tile_pool(name="accp", bufs=2))
    small = es.enter_context(tc.tile_pool(name="small", bufs=4))

    dram = es.enter_context(tc.tile_pool(name="dram", bufs=1, space="DRAM"))
    # tiny warm-up AllReduce: absorbs collective-comm setup under phase A
    ccw_in = dram.tile([1, 16], F32)
    ccw_out = dram.tile([1, 16], F32)
    cc_in = dram.tile([G, B * D], F32)
    cc_out = dram.tile([G, B * D], F32)
    warm_sb = singles.tile([1, 16], F32)
    nc.vector.memset(warm_sb[:], 0.0)
    nc.gpsimd.dma_start(ccw_in[:], warm_sb[:])
    nc.gpsimd.collective_compute(
        "AllReduce", add, replica_groups=[list(range(N_CORES))],
        ins=[ccw_in.opt()], outs=[ccw_out.opt()],
    )

    # ---------------- Phase A: stream + fp8 DoubleRow sums ------------------
    with tc.tile_pool(name="psumS", bufs=1, space="PSUM") as psS, \
         tc.tile_pool(name="psumT", bufs=1, space="PSUM") as psT, \
         tc.tile_pool(name="psumXA", bufs=3, space="PSUM") as psXA, \
         tc.tile_pool(name="psumU", bufs=2, space="PSUM") as psU, \
         tc.tile_pool(name="xtA", bufs=3) as xtA:
        sums_ps = psS.tile([2 * DB, H], F32)

        # w rides one pool ring slot briefly (freed after the transposes)
        w_sb = pool.tile([G, C], F32, tag="t", name="w_sb")
        nc.sync.dma_start(w_sb[:], w[:])
        # one-time W_u transpose: wT[c, g] chunks in bf16 (fixup) + fp8 (u)
        for k in range(NKC):
            tr = psT.tile([P, 2 * DB * (NKC // 2)], F32, tag="fix")
            nc.tensor.transpose(tr[:, :G], w_sb[:, k * P:(k + 1) * P],
                                ident[:G, :G])
            nc.scalar.copy(wT_bf[:, k, :], tr[:, :G])
            nc.vector.tensor_copy(wT_f8[:, k // 2, k % 2, :], tr[:, :G])

        # phase-A tile order: d7 tiles (which carry the u-path PE work)
        # spread 1-in-7 among the d0..d6 stream so PE load stays level
        others = [(d_, b_, j_) for d_ in range(D - 1)
                  for b_ in range(B) for j_ in range(nj)]
        d7s = [(D - 1, b_, j_) for b_ in range(B) for j_ in range(nj)]
        NT = DB * nj
        seq = []
        oi = si = 0
        for i in range(NT):
            if i % 7 == 3 and si < len(d7s):
                seq.append(d7s[si])
                si += 1
            else:
                seq.append(others[oi])
                oi += 1

        for ti, (d, b, j) in enumerate(seq):
            r = d * B + b
            t = pool.tile([P, C], F32, tag="t")
            nc.sync.dma_start(t[:], x[d, b, j * P:(j + 1) * P, :])
            q = f8p.tile([P, C], F8, tag="q")
            # casts split ~4:3 DVE:ACT (ACT's converter is slower)
            on_dve = ti % 7 in (0, 2, 4, 6)
            if on_dve:
                nc.vector.tensor_copy(q[:], t[:])
                if d in RESID:
                    nc.scalar.copy(xr[d][:, b, j, :], t[:])
            else:
                nc.scalar.copy(q[:], t[:])
                if d in RESID:
                    nc.vector.tensor_copy(xr[d][:, b, j, :], t[:])
            nc.tensor.matmul(
                sums_ps[:], ind_f8[:, r],
                q[:].rearrange("p (i h) -> p i h", i=2),
                start=(ti == 0), stop=(ti == NT - 1),
                perf_mode=mybir.MatmulPerfMode.DoubleRow)
            if d == D - 1:
                # u-path for block (b, j), all fp8: transpose the fp8 tile's
                # c-chunks on PE, DoubleRow uT += wT_f8.T @ x7T pairs
                u_ps = psU.tile([G, P], F32, tag="u")
                for kp in range(NKC // 2):
                    xt2 = xtA.tile([P, 2, P], F8, tag="xt2")
                    for i2 in range(2):
                        k = 2 * kp + i2
                        # fp8 transpose writes with element step 2
                        xt_ps = psXA.tile([P, P, 2], F8, tag="xt_ps")
                        nc.tensor.matmul(
                            xt_ps[:, :, 0], q[:, k * P:(k + 1) * P],
                            ident_f8[:], is_transpose=True)
                        if (kp + i2) % 2 == 0:
                            nc.scalar.copy(xt2[:, i2, :], xt_ps[:, :, 0])
                        else:
                            nc.vector.tensor_copy(xt2[:, i2, :],
                                                  xt_ps[:, :, 0])
                    nc.tensor.matmul(
                        u_ps[:], wT_f8[:, kp], xt2[:],
                        start=(kp == 0), stop=(kp == NKC // 2 - 1),
                        perf_mode=mybir.MatmulPerfMode.DoubleRow)
                nc.vector.tensor_copy(uT_bf[:, b, j, :], u_ps[:])

        # raw sums -> sbuf with the 1/S mean scale folded in (ACT)
        nc.scalar.mul(sums_sb[:], sums_ps[:], 1.0 / S)

        # ---- local partial keysT in bf16 (keys are linear in the means, ----
        # ---- so the AllReduce runs in the tiny keys space: 8KB)         ----
        # meanT chunks: 4 fp32 transposes [64,128] -> [128,64] into one psum
        mt_ps = psT.tile([P, 2 * DB * (NKC // 2)], F32, tag="fix")
        for kp in range(NKC // 2):
            nc.tensor.matmul(
                mt_ps[:, kp * 2 * DB:(kp + 1) * 2 * DB],
                sums_sb[:, kp * P:(kp + 1) * P], ident[:2 * DB, :2 * DB],
                is_transpose=True,
                start=(kp == 0), stop=(kp == NKC // 2 - 1))
        nc.vector.tensor_copy(meanT_bf[:], mt_ps[:])
        # partial keysT[g, d] per b = sum_(h,k') wT_(h*4+k').T @ meanT cols
        keys_ps = psT.tile([P, 2 * DB * (NKC // 2)], F32, tag="fix")
        for bb in range(B):
            for kk in range(NKC):
                h, kp = divmod(kk, NKC // 2)
                mcols = meanT_bf[:, kp * 2 * DB:(kp + 1) * 2 * DB].rearrange(
                    "p (dd m) -> p dd m", m=2 * B)[:, :, 2 * bb + h]
                nc.tensor.matmul(
                    keys_ps[:G, bb * D:(bb + 1) * D],
                    wT_bf[:, kk, :], mcols,
                    start=(kk == 0), stop=(kk == NKC - 1))
        nc.vector.tensor_copy(sumk_sb[:], keys_ps[:G, :B * D])

    # ---------------- AllReduce the [G, B*D] partial keys -------------------
    # bounce DMAs go through GpSimd's queue so the Sync engine never blocks
    # on the collective and keeps issuing phase-B prefetch reads.
    nc.gpsimd.dma_start(cc_in[:], sumk_sb[:])
    nc.gpsimd.collective_compute(
        "AllReduce", add,
        replica_groups=[list(range(N_CORES))],
        ins=[cc_in.opt()], outs=[cc_out.opt()],
    )
    nc.gpsimd.dma_start(keysT_f32[:], cc_out[:])
    nc.vector.tensor_copy(keysT_bf[:].rearrange("g b d -> g (b d)"),
                          keysT_f32[:])

    # ---------------- Phase B: gates + depth-weighted sum -------------------
    # Resident depths ride the PE: ACT builds diag(g_d) stationaries, PE
    # accumulates g_d * x_d for d in RESID into PSUM. DVE chains the
    # streamed depths (freeing pool slots ASAP post-collective) and merges
    # the PSUM partial at the end.
    with tc.tile_pool(name="psumL", bufs=2, space="PSUM") as psL, \
         tc.tile_pool(name="psumAcc", bufs=2, space="PSUM") as psA, \
         tc.tile_pool(name="diagp", bufs=3) as diagp:
        for b in range(B):
            for j in range(nj):
                ts = {}
                for sd in STREAMED:
                    tt = pool.tile([P, C], F32, tag="t")
                    nc.sync.dma_start(tt[:], x[sd, b, j * P:(j + 1) * P, :])
                    ts[sd] = tt
                # logits for this block: one small matmul off resident uT
                lg_ps = psL.tile([P, D], F32, tag="lg")
                nc.tensor.matmul(lg_ps[:], uT_bf[:, b, j, :],
                                 keysT_bf[:, b, :])
                e_sb = small.tile([P, D], F32, tag="e")
                z_sb = small.tile([P, 1], F32, tag="z")
                rz_sb = small.tile([P, 1], F32, tag="rz")
                nc.scalar.activation(
                    e_sb[:], lg_ps[:], mybir.ActivationFunctionType.Exp,
                    accum_out=z_sb[:])
                nc.vector.reciprocal(rz_sb[:], z_sb[:])
                nc.scalar.mul(gates_sb[:, b, j, :], e_sb[:], rz_sb[:])

                # resident partial on PE
                r_ps = psA.tile([P, C], F32, tag="racc")
                for di, d in enumerate(RESID):
                    dg = diagp.tile([P, P], BF16, tag="dg")
                    nc.scalar.mul(dg[:], ident_bf[:],
                                  gates_sb[:, b, j, d:d + 1])
                    for h in range(2):
                        nc.tensor.matmul(
                            r_ps[:, h * H:(h + 1) * H], dg[:],
                            xr[d][:, b, j, h * H:(h + 1) * H],
                            start=(di == 0), stop=(di == len(RESID) - 1))
                # streamed chain on DVE, psum partial merged at the end
                acc = accp.tile([P, C], F32, tag="acc")
                for si, d in enumerate(STREAMED):
                    g = gates_sb[:, b, j, d:d + 1]
                    if si == 0:
                        nc.vector.tensor_scalar_mul(acc[:], ts[d][:], g)
                    else:
                        nc.vector.scalar_tensor_tensor(
                            out=acc[:], in0=ts[d][:], scalar=g,
                            in1=acc[:], op0=mul, op1=add)
                nc.vector.scalar_tensor_tensor(
                    out=acc[:], in0=r_ps[:], scalar=1.0,
                    in1=acc[:], op0=mul, op1=add)
                # y writes via GpSimd (SWDGE): keeps Sync's in-order queue
                # free for prefetch reads
                nc.gpsimd.dma_start(y[b, j * P:(j + 1) * P, :], acc[:])

    es.close()


def build_nc(s_sh):
    nc = bacc.Bacc("TRN2", target_bir_lowering=False, debug=False,
                   num_devices=N_CORES)
    x_ap = nc.dram_tensor("x", [D, B, s_sh, C], F32, kind="ExternalInput").ap()
    w_ap = nc.dram_tensor("w", [G, C], F32, kind="ExternalInput").ap()
    y_ap = nc.dram_tensor("y", [B, s_sh, C], F32, kind="ExternalOutput").ap()
    with tile.TileContext(nc) as tc:
        build_body(tc, x_ap, w_ap, y_ap, s_sh)
    nc.compile()
    return nc


_NC_CACHE = {}


def _get_nc(s_sh):
    if s_sh not in _NC_CACHE:
        _NC_CACHE[s_sh] = build_nc(s_sh)
    return _NC_CACHE[s_sh]


def run(cached_states, W_u, trace=False, trace_cores=None):
    s_sh = S // N_CORES
    nc = _get_nc(s_sh)
    xs = np.asarray(cached_states, dtype=np.float32)
    ws = np.ascontiguousarray(np.asarray(W_u, dtype=np.float32))
    in_maps = []
    for i in range(N_CORES):
        sh = np.ascontiguousarray(xs[:, :, i * s_sh:(i + 1) * s_sh, :])
        in_maps.append({"x": sh, "w": ws})
    res = bass_utils.run_bass_kernel_spmd(
        nc, in_maps, core_ids=list(range(N_CORES)), trace=trace,
        trace_cores=trace_cores)
    out = np.empty((B, S, C), np.float32)
    for i in range(N_CORES):
        out[:, i * s_sh:(i + 1) * s_sh, :] = res.results[i]["y"]
    return out, res


def kernel(cached_states, W_u):
    out, _ = run(cached_states, W_u)
    return out
